# revision 11
# baseline (speedup 1.0000x reference)
"""CosineAttention Trainium2 kernel: 8-core SPMD (batch x seq-stripe data parallel).

B=4, S=2048, D=1024.  Core c: batch b=c//2, stripe v=c%2.
Q-rows per core: 512-row blocks {v, 3-v} ("E"=block v, "L"=block 3-v) ->
balanced causal work.  Full K/V projection per core (duplicated within the
batch pair).  All matmuls in float32r (full-rate fp32 PE mode).

Math per core:
  QT[e,s]  = Wq @ qT + bq   (transposed layout, e on partitions)
  Qn = QT * rsqrt(sum_e QT^2)[s]      (column scaling via K=1 bcast matmul)
  KT stored to DRAM, rk'[t] = rsqrt(1024 * sum_e KT^2)[t]  (= 1/(32*||K||))
  V[t,e] row-major to DRAM
  S_T[t,s] = KT_j.T-chunks @ Qn  ;  P = exp(rk'[t] * S_T) * mask01
  den[s] = ones.T @ P ;  OT[e,s] = sum_j V_j.T-chunks @ P_j
  Y[s,eo] = (OT.T @ WoT) * (1/den)[s] + bo
"""
import sys

sys.path.insert(0, "/opt/trn_rl_repo")
import numpy as np
import concourse.bass as bass
import concourse.tile as tile
from concourse import mybir
from concourse.bass_utils import run_bass_kernel_spmd

FP32 = mybir.dt.float32
FP32R = mybir.dt.float32r
AF = mybir.ActivationFunctionType

B, S, D = 4, 2048, 1024
NC8 = 8
NE, NL = 8, 16  # static t-block counts for groups E and L

_ctr = [0]


def _legalize_waits(nc, default_limit=1, matmul_limit=0):
    """Walrus S3 instruction structs have very few sync-wait slots: hoist
    excess on_wait entries into standalone EventSemaphore carriers."""
    f = nc.m.functions[0]
    for b in f.blocks:
        new_list = []
        changed = False
        for inst in b.instructions:
            si = getattr(inst, "sync_info", None)
            waits = list(si.on_wait) if si is not None and si.on_wait else []
            limit = matmul_limit if inst.opcode in ("Matmult", "Ldweights") else default_limit
            if len(waits) > limit:
                for w in waits[: len(waits) - limit]:
                    _ctr[0] += 1
                    ev = mybir.InstEventSemaphore(
                        name=f"I-waitfix-{_ctr[0]}",
                        engine=inst.engine,
                        ins=[],
                        outs=[],
                        sync_info=mybir.SyncInfo(on_wait=[w], on_update=[]),
                    )
                    nc.register_instruction(ev)
                    new_list.append(ev)
                si.on_wait = waits[len(waits) - limit :]
                changed = True
            new_list.append(inst)
        if changed:
            b.instructions = new_list


def _build(nc: bass.Bass):
    qT = nc.declare_dram_parameter("qT", [D, 1024], FP32, isOutput=False)
    kT = nc.declare_dram_parameter("kT", [D, S], FP32, isOutput=False)
    vT = nc.declare_dram_parameter("vT", [D, S], FP32, isOutput=False)
    wqT = nc.declare_dram_parameter("wqT", [D, D], FP32, isOutput=False)
    wkT = nc.declare_dram_parameter("wkT", [D, D], FP32, isOutput=False)
    wvT = nc.declare_dram_parameter("wvT", [D, D], FP32, isOutput=False)
    woT = nc.declare_dram_parameter("woT", [D, D], FP32, isOutput=False)
    bias4 = nc.declare_dram_parameter("bias4", [5, D], FP32, isOutput=False)
    maskm = nc.declare_dram_parameter("maskm", [NE + NL, 128, 512], FP32, isOutput=False)
    ydram = nc.declare_dram_parameter("y", [1024, D], FP32, isOutput=True)

    kt_dram = nc.dram_tensor("kt_i", [16, 128, 8, 128], FP32R)
    rk_d = nc.dram_tensor("rk_d", [S], FP32)
    den_d = nc.dram_tensor("den_d", [2, 512], FP32)
    v_dram = nc.dram_tensor("v_i", [8, 16, 128, 128], FP32R)

    with nc.allow_low_precision(reason="float32r views of fp32 data (same bits)"), tile.TileContext(nc) as tc:
        import contextlib

        with contextlib.ExitStack() as st:
            consts = st.enter_context(tc.tile_pool(name="consts", bufs=1))
            accp = st.enter_context(tc.tile_pool(name="accp", bufs=3, space="PSUM"))
            ssqp = st.enter_context(tc.tile_pool(name="ssqp", bufs=1, space="PSUM"))
            sps = st.enter_context(tc.tile_pool(name="sps", bufs=2, space="PSUM"))
            otp = st.enter_context(tc.tile_pool(name="otp", bufs=2, space="PSUM"))
            qtp = st.enter_context(tc.tile_pool(name="qtp", bufs=1))
            bounce = st.enter_context(tc.tile_pool(name="bounce", bufs=3))
            sqp = st.enter_context(tc.tile_pool(name="sqp", bufs=2))
            normp = st.enter_context(tc.tile_pool(name="normp", bufs=1))

            ones_col = consts.tile([128, 1], FP32R, name="ones_col")
            nc.sync.dma_start(out=ones_col,
                              in_=bias4[4:5, 0:128].rearrange("a t -> t a").bitcast(FP32R))
            b4 = consts.tile([1, 5, D], FP32R, name="b4")
            for i in range(5):
                nc.sync.dma_start(out=b4[0:1, i, :], in_=bias4[i : i + 1, :].bitcast(FP32R))
            ones_row = b4[0:1, 4, 0:512]

            # bo broadcast [128, D]
            bo_rep = consts.tile([128, D], FP32, name="bo_rep")
            for h in range(2):
                bp = accp.tile([128, 512], FP32, name="bp", tag="acc")
                nc.tensor.matmul(bp, ones_row[0:1, 0:128], b4[0:1, 3, 512 * h : 512 * h + 512],
                                 start=True, stop=True)
                nc.vector.tensor_copy(out=bo_rep[:, 512 * h : 512 * h + 512], in_=bp)

            QTn = qtp.tile([128, 8, 1024], FP32R, name="QTn")
            rq_row = normp.tile([1, 1024], FP32R, name="rq_row")
            rk_row = normp.tile([1, S], FP32, name="rk_row")
            rkcol = normp.tile([128, 16], FP32, name="rkcol")
            den_sb = normp.tile([1, 2, 512], FP32, name="den_sb")
            rden_col = normp.tile([128, 2, 4], FP32, name="rden_col")

            # ---------------- P1: Q projection (transposed) + normalize -----
            with tc.tile_pool(name="wxp", bufs=1) as wxp, tc.tile_pool(name="xinp", bufs=1) as xinp:
                wq = wxp.tile([128, 8, D], FP32R, name="wq", tag="W")
                for ci in range(8):
                    nc.sync.dma_start(out=wq[:, ci, :], in_=wqT[128 * ci : 128 * ci + 128, :].bitcast(FP32R))
                xq = xinp.tile([128, 8, S], FP32R, name="xq", tag="xin")
                for ci in range(8):
                    nc.sync.dma_start(out=xq[:, ci, 0:1024], in_=qT[128 * ci : 128 * ci + 128, :].bitcast(FP32R))

                for stl in range(2):
                    c0 = 512 * stl
                    ssq = ssqp.tile([1, 512], FP32, name=f"ssqq{stl}", tag="ssq")
                    for m in range(8):
                        ps = accp.tile([128, 512], FP32, name="qps", tag="acc")
                        for ci in range(8):
                            nc.tensor.matmul(ps, wq[:, ci, 128 * m : 128 * m + 128],
                                             xq[:, ci, c0 : c0 + 512], start=(ci == 0), stop=False)
                        nc.tensor.matmul(ps, b4[0:1, 0, 128 * m : 128 * m + 128], ones_row,
                                         start=False, stop=True)
                        nc.vector.tensor_copy(out=QTn[:, m, c0 : c0 + 512], in_=ps)
                        sq = sqp.tile([128, 512], FP32R, name="sq", tag="sq")
                        nc.vector.tensor_mul(out=sq, in0=QTn[:, m, c0 : c0 + 512], in1=QTn[:, m, c0 : c0 + 512])
                        nc.tensor.matmul(ssq, ones_col, sq, start=(m == 0), stop=(m == 7))
                    nc.scalar.activation(out=rq_row[0:1, c0 : c0 + 512], in_=ssq, func=AF.Sqrt)
                    nc.vector.reciprocal(out=rq_row[0:1, c0 : c0 + 512], in_=rq_row[0:1, c0 : c0 + 512])
                for stl in range(2):
                    c0 = 512 * stl
                    rep = accp.tile([128, 512], FP32, name="rep", tag="acc")
                    nc.tensor.matmul(rep, ones_row[0:1, 0:128], rq_row[0:1, c0 : c0 + 512],
                                     start=True, stop=True)
                    for m in range(8):
                        nc.vector.tensor_mul(out=QTn[:, m, c0 : c0 + 512],
                                             in0=QTn[:, m, c0 : c0 + 512], in1=rep)

                # ---------------- P2: K projection -> DRAM + rk' ------------
                wk = wxp.tile([128, 8, D], FP32R, name="wk", tag="W")
                for ci in range(8):
                    nc.sync.dma_start(out=wk[:, ci, :], in_=wkT[128 * ci : 128 * ci + 128, :].bitcast(FP32R))
                xk = xinp.tile([128, 8, S], FP32R, name="xk", tag="xin")
                for ci in range(8):
                    nc.sync.dma_start(out=xk[:, ci, :], in_=kT[128 * ci : 128 * ci + 128, :].bitcast(FP32R))
                for stl in range(4):
                    c0 = 512 * stl
                    ssk = ssqp.tile([1, 512], FP32, name=f"ssk{stl}", tag="ssq")
                    for m in range(8):
                        ps = accp.tile([128, 512], FP32, name="kps", tag="acc")
                        for ci in range(8):
                            nc.tensor.matmul(ps, wk[:, ci, 128 * m : 128 * m + 128],
                                             xk[:, ci, c0 : c0 + 512], start=(ci == 0), stop=False)
                        nc.tensor.matmul(ps, b4[0:1, 1, 128 * m : 128 * m + 128], ones_row,
                                         start=False, stop=True)
                        kb = bounce.tile([128, 4, 128], FP32R, name="kb", tag="kb")
                        nc.vector.tensor_copy(out=kb, in_=ps.rearrange("p (j t) -> p j t", j=4))
                        sq = sqp.tile([128, 512], FP32R, name="sqk", tag="sq")
                        nc.vector.tensor_mul(out=sq, in0=kb.rearrange("p j t -> p (j t)"),
                                             in1=kb.rearrange("p j t -> p (j t)"))
                        nc.tensor.matmul(ssk, ones_col, sq, start=(m == 0), stop=(m == 7))
                        for jj in range(4):
                            nc.sync.dma_start(out=kt_dram[4 * stl + jj, :, m, :], in_=kb[:, jj, :])
                    nc.scalar.activation(out=rk_row[0:1, c0 : c0 + 512], in_=ssk, func=AF.Sqrt,
                                         scale=float(D))
                    nc.vector.reciprocal(out=rk_row[0:1, c0 : c0 + 512], in_=rk_row[0:1, c0 : c0 + 512])
                nc.sync.dma_start(out=rk_d[:].rearrange("(a s) -> a s", a=1), in_=rk_row[0:1, :])
                nc.sync.dma_start(out=rkcol, in_=rk_d[:].rearrange("(j t) -> t j", t=128))

                # ---------------- P3: V projection (row-major) -> DRAM ------
                wv = wxp.tile([128, 8, D], FP32R, name="wv", tag="W")
                for ci in range(8):
                    nc.sync.dma_start(out=wv[:, ci, :], in_=wvT[128 * ci : 128 * ci + 128, :].bitcast(FP32R))
                xv = xinp.tile([128, 8, S], FP32R, name="xv", tag="xin")
                for ci in range(8):
                    nc.sync.dma_start(out=xv[:, ci, :], in_=vT[128 * ci : 128 * ci + 128, :].bitcast(FP32R))
                for tcn in range(16):
                    for h in range(2):
                        ps = accp.tile([128, 512], FP32, name="vps", tag="acc")
                        for ci in range(8):
                            nc.tensor.matmul(ps, xv[:, ci, 128 * tcn : 128 * tcn + 128],
                                             wv[:, ci, 512 * h : 512 * h + 512], start=(ci == 0), stop=False)
                        nc.tensor.matmul(ps, ones_row[0:1, 0:128], b4[0:1, 2, 512 * h : 512 * h + 512],
                                         start=False, stop=True)
                        vb = bounce.tile([128, 4, 128], FP32R, name="vb", tag="kb")
                        nc.vector.tensor_copy(out=vb, in_=ps.rearrange("p (c e) -> p c e", c=4))
                        for cl in range(4):
                            nc.sync.dma_start(out=v_dram[4 * h + cl, tcn], in_=vb[:, cl, :])

            # ---------------- P4: attention + out-projection ----------------
            with (
                tc.tile_pool(name="wop", bufs=1) as wop,
                tc.tile_pool(name="ptp", bufs=1) as ptp,
                tc.tile_pool(name="kst", bufs=2) as kst,
                tc.tile_pool(name="vst", bufs=3) as vst,
                tc.tile_pool(name="mst", bufs=2) as mst,
                tc.tile_pool(name="otsb", bufs=1) as otsb,
                tc.tile_pool(name="ysb", bufs=2) as ysb,
            ):
                wo = wop.tile([128, 8, D], FP32R, name="wo")
                for ci in range(8):
                    nc.sync.dma_start(out=wo[:, ci, :], in_=woT[128 * ci : 128 * ci + 128, :].bitcast(FP32R))

                groups = [("L", NL, 512, 0), ("E", NE, 0, NL)]  # (name, nT, qcol, mask_off)
                for gname, nT, qc, moff in groups:
                    gidx = 0 if gname == "L" else 1
                    PT = ptp.tile([128, NL, 512], FP32R, name=f"PT{gname}", tag="pt")
                    # phase A: scores -> exp -> mask; den
                    for j in range(nT):
                        ktile = kst.tile([128, 8, 128], FP32R, name="ktile", tag="kt")
                        nc.sync.dma_start(out=ktile, in_=kt_dram[j])
                        sp = sps.tile([128, 512], FP32, name="sp", tag="sp")
                        for ci in range(8):
                            nc.tensor.matmul(sp, ktile[:, ci, :], QTn[:, ci, qc : qc + 512],
                                             start=(ci == 0), stop=(ci == 7))
                        nc.scalar.activation(out=PT[:, j, :], in_=sp, func=AF.Exp,
                                             scale=rkcol[:, j : j + 1])
                        mt = mst.tile([128, 512], FP32, name="mt", tag="mt")
                        nc.sync.dma_start(out=mt, in_=maskm[moff + j])
                        nc.vector.tensor_mul(out=PT[:, j, :], in0=PT[:, j, :], in1=mt)
                        dp = sps.tile([128, 512], FP32, name="dp", tag="sp")
                        nc.tensor.matmul(dp[0:1, :], ones_col, PT[:, j, :], start=True, stop=True)
                        if j == 0:
                            nc.vector.tensor_copy(out=den_sb[0:1, gidx, :], in_=dp[0:1, :])
                        else:
                            nc.vector.tensor_add(out=den_sb[0:1, gidx, :], in0=den_sb[0:1, gidx, :],
                                                 in1=dp[0:1, :])
                    # phase B: OT accumulation
                    OT = otsb.tile([128, 8, 512], FP32R, name=f"OT{gname}", tag="ot")
                    for ci in range(8):
                        op = otp.tile([128, 512], FP32, name="op", tag="op")
                        for j in range(nT):
                            vsl = vst.tile([128, 128], FP32R, name="vsl", tag="vs")
                            nc.sync.dma_start(out=vsl, in_=v_dram[ci, j])
                            nc.tensor.matmul(op, vsl, PT[:, j, :], start=(j == 0), stop=(j == nT - 1))
                        nc.vector.tensor_copy(out=OT[:, ci, :], in_=op)
                    # rden
                    nc.vector.reciprocal(out=den_sb[0:1, gidx, :], in_=den_sb[0:1, gidx, :])
                    nc.sync.dma_start(out=den_d[gidx : gidx + 1, :], in_=den_sb[0:1, gidx, :])
                    nc.sync.dma_start(out=rden_col[:, gidx, :],
                                      in_=den_d[gidx].rearrange("(sb t) -> t sb", t=128))
                    # out-projection
                    rowbase = 512 * (0 if gname == "E" else 1)
                    for sb in range(4):
                        for h in range(2):
                            yp = accp.tile([128, 512], FP32, name="yp", tag="acc")
                            for ci in range(8):
                                nc.tensor.matmul(yp, OT[:, ci, 128 * sb : 128 * sb + 128],
                                                 wo[:, ci, 512 * h : 512 * h + 512],
                                                 start=(ci == 0), stop=(ci == 7))
                            yt = ysb.tile([128, 512], FP32, name="yt", tag="yt")
                            nc.vector.tensor_scalar_mul(yt, yp, rden_col[:, gidx, sb : sb + 1])
                            nc.vector.tensor_add(out=yt, in0=yt, in1=bo_rep[:, 512 * h : 512 * h + 512])
                            nc.sync.dma_start(
                                out=ydram[rowbase + 128 * sb : rowbase + 128 * sb + 128,
                                          512 * h : 512 * h + 512], in_=yt)
    _legalize_waits(nc)
    return nc


_cache = {}


def _get_nc():
    if "nc" not in _cache:
        nc = bass.Bass(trn_type="TRN2")
        _build(nc)
        _cache["nc"] = nc
    return _cache["nc"]


def kernel(q, k, v, Wq, bq, Wk, bk, Wv, bv, Wo, bo, attn_mask, key_padding_mask):
    q = np.asarray(q, np.float32)
    k = np.asarray(k, np.float32)
    v = np.asarray(v, np.float32)
    am = np.asarray(attn_mask, bool)
    kp = np.asarray(key_padding_mask, bool)

    wqT = np.ascontiguousarray(np.asarray(Wq, np.float32).T)
    wkT = np.ascontiguousarray(np.asarray(Wk, np.float32).T)
    wvT = np.ascontiguousarray(np.asarray(Wv, np.float32).T)
    woT = np.ascontiguousarray(np.asarray(Wo, np.float32).T)
    bias4 = np.stack([np.asarray(bq, np.float32), np.asarray(bk, np.float32),
                      np.asarray(bv, np.float32), np.asarray(bo, np.float32),
                      np.ones(D, np.float32)])

    nc = _get_nc()
    kT_by_b = [np.ascontiguousarray(k[b].T) for b in range(B)]
    vT_by_b = [np.ascontiguousarray(v[b].T) for b in range(B)]
    in_maps = []
    meta = []
    for c in range(NC8):
        b, vv = divmod(c, 2)
        rows_E = np.arange(512 * vv, 512 * vv + 512)
        rows_L = np.arange(512 * (3 - vv), 512 * (3 - vv) + 512)
        # kernel rows: [0:512]=E, [512:1024]=L ; qT columns [E | L]
        rows = np.concatenate([rows_E, rows_L])
        qT = np.ascontiguousarray(q[b][rows].T)
        kTb = kT_by_b[b]
        vTb = vT_by_b[b]
        # mask tiles: group L occupies slots [0:NL], group E slots [NL:NL+NE]
        maskm = np.zeros((NE + NL, 128, 512), np.float32)
        full = ~(am | kp[b][None, :])  # [S_q, S_k] True = keep
        for j in range(NL):
            blk = full[rows_L, 128 * j : 128 * j + 128]  # [512, 128]
            maskm[j] = blk.T.astype(np.float32)
        for j in range(NE):
            blk = full[rows_E, 128 * j : 128 * j + 128]
            maskm[NL + j] = blk.T.astype(np.float32)
        # correctness guard: q-rows of group E must not attend beyond NE*128
        assert not full[rows_E, NE * 128 :].any(), "mask extends beyond static E window"
        in_maps.append({
            "qT": qT, "kT": kTb, "vT": vTb,
            "wqT": wqT, "wkT": wkT, "wvT": wvT, "woT": woT,
            "bias4": bias4, "maskm": maskm,
        })
        meta.append((b, rows_E, rows_L))

    res = run_bass_kernel_spmd(nc, in_maps, list(range(NC8)))
    out = np.empty((B, S, D), np.float32)
    for c in range(NC8):
        b, rows_E, rows_L = meta[c]
        y = res.results[c]["y"]
        out[b, rows_E] = y[0:512]
        out[b, rows_L] = y[512:1024]
    return out
